# revision 1
# baseline (speedup 1.0000x reference)
"""nn_CrossAttention kernel for 8x TRN2 NeuronCores.

Sharding: core c = (batch b = c//2, head-group hg = c%2 of 8 heads).
Each core: projections (f32r matmuls), scoresT = K^T-layout QK^T with
2-head PE row-packing, exp on ACT (scale 1/8 fused), A*V with a
ones-augmented V (extra output row = softmax denominator), normalize via
K=1 broadcast matmul + DVE multiply. Host pre-transposes activations and
splits W columns per head-group; host re-assembles the [512,2048] per-core
ctxT outputs into the full [4,2048,1024] output.
"""

import json
import numpy as np

B, S, D, NH, HD = 4, 2048, 1024, 16, 64
CPC = 512          # cols per core = 8 heads * 64
NCORES = 8
NDT = D // 128     # 8 d-tiles
NP = CPC // 128    # 4 c-tiles (head pairs)
NSK = S // 128     # 16 sk-tiles
NJ = S // 512      # 4 sq chunks
SQC = 512          # sq chunk size


# ---------------------------------------------------------------- drain fix
def _fix_module_json(bj: bytes) -> bytes:
    """This walrus build accepts at most ONE sync wait/update on CTRL-lowered
    instructions (Drain). Move extras onto EventSemaphore instructions."""
    d = json.loads(bj)
    counter = [0]

    def fix_block(b):
        out = []
        for inst in b.get("instructions", []):
            si = inst.get("sync_info") or {}
            ow = si.get("on_wait") or []
            ou = si.get("on_update") or []
            if (inst.get("opcode") not in
                    ("EventSemaphore", "Call", "RegisterMove",
                     "UnconditionalBranch", "ISA", "Drain") and len(ow) > 1):
                # Several instruction structs in this walrus have room for
                # only one sync wait; hoist extras onto EventSemaphores
                # issued just before on the same engine (engine streams are
                # serial, so the blocking point is identical). Updates stay.
                for w in ow[1:]:
                    counter[0] += 1
                    out.append({
                        "debug": inst.get("debug", 0),
                        "engine": inst["engine"],
                        "ins": [], "outs": [],
                        "name": f"synthmmw-{counter[0]}",
                        "opcode": "EventSemaphore",
                        "sync_info": {"on_update": [], "on_wait": [w]},
                    })
                inst["sync_info"] = {"on_update": ou, "on_wait": ow[:1]}
                out.append(inst)
                continue
            if inst.get("opcode") == "Drain" and (len(ow) > 1 or len(ou) > 1):
                for w in ow[1:]:
                    counter[0] += 1
                    out.append({
                        "debug": inst.get("debug", 0),
                        "engine": inst["engine"],
                        "ins": [], "outs": [],
                        "name": f"synthwait-{counter[0]}",
                        "opcode": "EventSemaphore",
                        "sync_info": {"on_update": [], "on_wait": [w]},
                    })
                inst["sync_info"] = {"on_update": ou[:1], "on_wait": ow[:1]}
                out.append(inst)
                for u in ou[1:]:
                    counter[0] += 1
                    out.append({
                        "debug": inst.get("debug", 0),
                        "engine": inst["engine"],
                        "ins": [], "outs": [],
                        "name": f"synthupd-{counter[0]}",
                        "opcode": "EventSemaphore",
                        "sync_info": {"on_update": [u], "on_wait": []},
                    })
            else:
                out.append(inst)
        b["instructions"] = out
        for sb in b.get("blocks", []):
            fix_block(sb)

    for fn in d.get("functions", []):
        for blk in fn.get("blocks", []):
            fix_block(blk)
    return json.dumps(d).encode()


def _install_drainfix():
    import concourse.bass as bass
    if getattr(bass.Bass, "_drainfix_installed", False):
        return
    orig = bass.Bass.to_json_bytes

    def patched(self):
        return _fix_module_json(orig(self))

    bass.Bass.to_json_bytes = patched
    bass.Bass._drainfix_installed = True


# ---------------------------------------------------------------- program
_NC_CACHE = []


def _build_nc(reps=1):
    import concourse.bass as bass
    import concourse.mybir as mybir
    from concourse.tile import TileContext
    from contextlib import ExitStack

    f32 = mybir.dt.float32
    f32r = mybir.dt.float32r
    bf16 = mybir.dt.bfloat16
    EXP = mybir.ActivationFunctionType.Exp

    nc = bass.Bass("TRN2", num_devices=NCORES)

    xqT = nc.dram_tensor("xqT", [D, S], f32, kind="ExternalInput")
    xkT = nc.dram_tensor("xkT", [D, S], f32, kind="ExternalInput")
    xvT = nc.dram_tensor("xvT", [D, S], f32, kind="ExternalInput")
    wq = nc.dram_tensor("wq", [D, CPC], f32, kind="ExternalInput")
    wk = nc.dram_tensor("wk", [D, CPC], f32, kind="ExternalInput")
    wv = nc.dram_tensor("wv", [D, CPC], f32, kind="ExternalInput")
    bqd = nc.dram_tensor("bq", [CPC], f32, kind="ExternalInput")
    bkd = nc.dram_tensor("bk", [CPC], f32, kind="ExternalInput")
    bvd = nc.dram_tensor("bv", [CPC], f32, kind="ExternalInput")
    conesd = nc.dram_tensor("cones", [1, 64], f32, kind="ExternalInput")
    outd = nc.dram_tensor("out", [CPC, S], f32, kind="ExternalOutput")

    with ExitStack() as ctx:
        ctx.enter_context(nc.allow_low_precision(
            reason="f32r tiles are full fp32 storage; matmul accumulates f32"))
        tc = ctx.enter_context(TileContext(nc))
        sb = ctx.enter_context(tc.tile_pool(name="sb", bufs=1))
        ps = ctx.enter_context(tc.tile_pool(name="ps", bufs=1, space="PSUM"))

        # ---- constants ----
        bq_sb = sb.tile([128, NP], f32, name="bq_sb")
        nc.sync.dma_start(out=bq_sb, in_=bqd.rearrange("(p c) -> c p", p=NP))
        bk_sb = sb.tile([128, NP], f32, name="bk_sb")
        nc.sync.dma_start(out=bk_sb, in_=bkd.rearrange("(p c) -> c p", p=NP))
        bv_bc = sb.tile([128, CPC], f32, name="bv_bc")
        _bva = bvd[:]
        nc.sync.dma_start(
            out=bv_bc,
            in_=bass.AP(tensor=_bva.tensor, offset=_bva.offset,
                        ap=[[0, 128]] + list(_bva.ap)))
        ones = sb.tile([1, 64], f32r, name="ones")
        nc.sync.dma_start(out=ones, in_=conesd[:, :].bitcast(f32r))

        # wv resident [128, dd, 512]
        wv_sb = sb.tile([128, NDT, CPC], f32r, name="wv_sb")
        for dd in range(NDT):
            nc.sync.dma_start(out=wv_sb[:, dd, :], in_=wv[dd * 128:(dd + 1) * 128, :].bitcast(f32r))

        # resident qT/kT [c 128, s 2048] per head pair
        qT = [sb.tile([128, S], f32r, name=f"qT{p}") for p in range(NP)]
        kT = [sb.tile([128, S], f32r, name=f"kT{p}") for p in range(NP)]
        # V augmented with ones column, bf16: [sk_in_tile, sk_tile, head, 65]
        v_aug = sb.tile([128, NSK, 8, 65], bf16, name="v_aug")
        nc.gpsimd.memset(v_aug[:, :, :, 64:65], 1.0)

        # ---- helpers ----
        def emit_proj_qk(t, plist, w_dram, x_dram, bias_sb, dst):
            """dst[p][:, t*512:(t+1)*512] = (x @ W + b).T chunk; contract D."""
            xts, wts = [], []
            for dd in range(NDT):
                x_t = sb.tile([128, SQC], f32r, tag="xs", bufs=10, name=f"x_{t}_{dd}")
                nc.sync.dma_start(
                    out=x_t, in_=x_dram[dd * 128:(dd + 1) * 128, t * SQC:(t + 1) * SQC].bitcast(f32r))
                xts.append(x_t)
                c0, c1 = plist[0] * 128, (plist[-1] + 1) * 128
                w_t = sb.tile([128, c1 - c0], f32r, tag="ws", bufs=10, name=f"w_{t}_{dd}")
                nc.sync.dma_start(out=w_t, in_=w_dram[dd * 128:(dd + 1) * 128, c0:c1].bitcast(f32r))
                wts.append(w_t)
            for p in plist:
                pr = ps.tile([128, SQC], f32, tag="vp", bufs=2, name=f"prj_{t}_{p}")
                off = (p - plist[0]) * 128
                for dd in range(NDT):
                    nc.tensor.matmul(
                        pr[:, :],
                        wts[dd][:, off:off + 128],
                        xts[dd][:, :],
                        start=(dd == 0), stop=(dd == NDT - 1))
                nc.vector.tensor_scalar_add(
                    dst[p][:, t * SQC:(t + 1) * SQC], pr[:, :], bias_sb[:, p:p + 1])

        def emit_proj_v(tt_list):
            """v_aug[:, tt, h, 0:64] = (xv @ Wv + bv) rows tt*128.., bf16."""
            for tt in tt_list:
                xvt = []
                for dd in range(NDT):
                    xv_t = sb.tile([128, 128], f32r, tag="xv", bufs=6, name=f"xv_{tt}_{dd}")
                    nc.sync.dma_start(
                        out=xv_t,
                        in_=xvT[dd * 128:(dd + 1) * 128, tt * 128:(tt + 1) * 128].bitcast(f32r))
                    xvt.append(xv_t)
                pv = ps.tile([128, CPC], f32, tag="vp", bufs=2, name=f"pv_{tt}")
                for dd in range(NDT):
                    nc.tensor.matmul(
                        pv[:, :],
                        xvt[dd][:, :],
                        wv_sb[:, dd, :],
                        start=(dd == 0), stop=(dd == NDT - 1))
                nc.vector.tensor_add(
                    v_aug[:, tt, :, 0:64],
                    pv.rearrange("c (h d) -> c h d", h=8),
                    bv_bc.rearrange("c (h d) -> c h d", h=8))

        alpha = {}  # (p, j, h, g) -> tile [128, 2, 512] bf16 (sk pair g)

        def emit_qkexp(p, j):
            """scoresT + exp for pair p, sq chunk j. sk-tiles in pairs g."""
            for g in range(NSK // 2):
                sc = []
                for h in range(2):
                    s_h = ps.tile([128, 2, SQC], f32, tag="scores", bufs=2,
                                  name=f"sc_{p}_{j}_{g}_{h}")
                    sc.append(s_h)
                for u in range(2):  # sk-tile i = 2g+u
                    i = 2 * g + u
                    for h in range(2):
                        nc.tensor.matmul(
                            sc[h][:, u, :],
                            kT[p][h * 64:(h + 1) * 64, i * 128:(i + 1) * 128],
                            qT[p][h * 64:(h + 1) * 64, j * SQC:(j + 1) * SQC],
                            start=True, stop=True)
                for h in range(2):
                    a_t = sb.tile([128, 2, SQC], bf16, tag="alpha", bufs=16,
                                  name=f"al_{p}_{j}_{g}_{h}")
                    nc.scalar.activation(a_t[:, :, :], sc[h][:, :, :], EXP, scale=0.125)
                    alpha[(p, j, h, g)] = a_t

        def emit_av(p, j):
            """ctxT rows for pair p chunk j: accumulate over sk, normalize, out."""
            for h in range(2):
                av = ps.tile([65, SQC], f32, tag="av", bufs=2, name=f"av_{p}_{j}_{h}")
                for g in range(NSK // 2):
                    a_t = alpha.pop((p, j, h, g))
                    for u in range(2):
                        i = 2 * g + u
                        nc.tensor.matmul(
                            av[:, :],
                            v_aug[:, i, 2 * p + h, :],
                            a_t[:, u, :],
                            start=(i == 0), stop=(i == NSK - 1))
                rec = sb.tile([1, SQC], f32r, tag="rec", bufs=4, name=f"rec_{p}_{j}_{h}")
                nc.vector.reciprocal(rec[:, :], av[64:65, :])
                bc = ps.tile([64, SQC], f32, tag="vp", bufs=2, name=f"bc_{p}_{j}_{h}")
                nc.tensor.matmul(bc[:, :], ones[:, :],
                                 rec[:, :], start=True, stop=True)
                bcs = sb.tile([64, SQC], f32, tag="bcs", bufs=4, name=f"bcs_{p}_{j}_{h}")
                nc.vector.tensor_copy(bcs[:, :], bc[:, :])
                cx = sb.tile([64, SQC], f32, tag="cx", bufs=4, name=f"cx_{p}_{j}_{h}")
                nc.vector.tensor_mul(cx[:, :], av[0:64, :], bcs[:, :])
                r0 = (2 * p + h) * 64
                nc.sync.dma_start(
                    out=outd[r0:r0 + 64, j * SQC:(j + 1) * SQC], in_=cx[:, :])

        # ---- emission schedule ----
        def _emit_all():
            for t in range(NJ):
                emit_proj_qk(t, [0], wk, xkT, bk_sb, kT)
            for t in range(NJ):
                emit_proj_qk(t, [0], wq, xqT, bq_sb, qT)
            emit_qkexp(0, 0)
            emit_proj_v(range(NSK))
            emit_qkexp(0, 1)
            emit_av(0, 0)
            for t in range(NJ):
                emit_proj_qk(t, [1, 2, 3], wk, xkT, bk_sb, kT)
            emit_qkexp(0, 2)
            emit_av(0, 1)
            for t in range(NJ):
                emit_proj_qk(t, [1, 2, 3], wq, xqT, bq_sb, qT)
            emit_qkexp(0, 3)
            emit_av(0, 2)
            seq = [(p, j) for p in range(NP) for j in range(NJ)]
            prev = [(0, 3)]
            for (p, j) in seq[4:]:
                emit_qkexp(p, j)
                emit_av(*prev.pop(0))
                prev.append((p, j))
            for pj in prev:
                emit_av(*pj)

        for _rep in range(reps):
            _emit_all()

    return nc


_NC_BY_REPS = {}


def _get_nc(reps=1):
    if reps not in _NC_BY_REPS:
        _install_drainfix()
        _NC_BY_REPS[reps] = _build_nc(reps)
    return _NC_BY_REPS[reps]


# ---------------------------------------------------------------- entry
def kernel(query, key_in, value, Wq, bq, Wk, bk, Wv, bv):
    from concourse.bass_utils import run_bass_kernel_spmd

    nc = _get_nc()
    query = np.asarray(query, np.float32)
    key_in = np.asarray(key_in, np.float32)
    value = np.asarray(value, np.float32)
    Wq = np.asarray(Wq, np.float32)
    Wk = np.asarray(Wk, np.float32)
    Wv = np.asarray(Wv, np.float32)
    bq = np.asarray(bq, np.float32)
    bk = np.asarray(bk, np.float32)
    bv = np.asarray(bv, np.float32)

    in_maps = []
    for c in range(NCORES):
        b, hg = divmod(c, 2)
        cols = slice(hg * CPC, (hg + 1) * CPC)
        in_maps.append({
            "xqT": np.ascontiguousarray(query[b].T),
            "xkT": np.ascontiguousarray(key_in[b].T),
            "xvT": np.ascontiguousarray(value[b].T),
            "wq": np.ascontiguousarray(Wq[:, cols]),
            "wk": np.ascontiguousarray(Wk[:, cols]),
            "wv": np.ascontiguousarray(Wv[:, cols]),
            "bq": np.ascontiguousarray(bq[cols]),
            "bk": np.ascontiguousarray(bk[cols]),
            "bv": np.ascontiguousarray(bv[cols]),
            "cones": np.ones((1, 64), np.float32),
        })

    res = run_bass_kernel_spmd(nc, in_maps, core_ids=list(range(NCORES)))

    out = np.empty((B, S, D), np.float32)
    for c in range(NCORES):
        b, hg = divmod(c, 2)
        out[b, :, hg * CPC:(hg + 1) * CPC] = res.results[c]["out"].T
    return out



# revision 21
# speedup vs baseline: 1.4333x; 1.4333x over previous
"""nn_CrossAttention kernel for 8x TRN2 NeuronCores.

Sharding: core c = (batch b = c//2, head-group hg = c%2 of 8 heads).
Each core: projections (f32r matmuls), scoresT = K^T-layout QK^T with
2-head PE row-packing, exp on ACT (scale 1/8 fused), A*V with a
ones-augmented V (extra output row = softmax denominator), normalize via
K=1 broadcast matmul + DVE multiply. Host pre-transposes activations and
splits W columns per head-group; host re-assembles the [512,2048] per-core
ctxT outputs into the full [4,2048,1024] output.
"""

import json
import numpy as np

B, S, D, NH, HD = 4, 2048, 1024, 16, 64
CPC = 512          # cols per core = 8 heads * 64
NCORES = 8
NDT = D // 128     # 8 d-tiles
NP = CPC // 128    # 4 c-tiles (head pairs)
NSK = S // 128     # 16 sk-tiles
NJ = S // 512      # 4 sq chunks
SQC = 512          # sq chunk size


# ---------------------------------------------------------------- drain fix
def _fix_module_json(bj: bytes) -> bytes:
    """This walrus build accepts at most ONE sync wait/update on CTRL-lowered
    instructions (Drain). Move extras onto EventSemaphore instructions."""
    d = json.loads(bj)
    counter = [0]

    def fix_block(b):
        out = []
        for inst in b.get("instructions", []):
            si = inst.get("sync_info") or {}
            ow = si.get("on_wait") or []
            ou = si.get("on_update") or []
            if (inst.get("opcode") not in
                    ("EventSemaphore", "Call", "RegisterMove",
                     "UnconditionalBranch", "ISA", "Drain") and len(ow) > 1):
                # Several instruction structs in this walrus have room for
                # only one sync wait; hoist extras onto EventSemaphores
                # issued just before on the same engine (engine streams are
                # serial, so the blocking point is identical). Updates stay.
                for w in ow[1:]:
                    counter[0] += 1
                    out.append({
                        "debug": inst.get("debug", 0),
                        "engine": inst["engine"],
                        "ins": [], "outs": [],
                        "name": f"synthmmw-{counter[0]}",
                        "opcode": "EventSemaphore",
                        "sync_info": {"on_update": [], "on_wait": [w]},
                    })
                inst["sync_info"] = {"on_update": ou, "on_wait": ow[:1]}
                out.append(inst)
                continue
            if inst.get("opcode") == "Drain" and (len(ow) > 1 or len(ou) > 1):
                for w in ow[1:]:
                    counter[0] += 1
                    out.append({
                        "debug": inst.get("debug", 0),
                        "engine": inst["engine"],
                        "ins": [], "outs": [],
                        "name": f"synthwait-{counter[0]}",
                        "opcode": "EventSemaphore",
                        "sync_info": {"on_update": [], "on_wait": [w]},
                    })
                inst["sync_info"] = {"on_update": ou[:1], "on_wait": ow[:1]}
                out.append(inst)
                for u in ou[1:]:
                    counter[0] += 1
                    out.append({
                        "debug": inst.get("debug", 0),
                        "engine": inst["engine"],
                        "ins": [], "outs": [],
                        "name": f"synthupd-{counter[0]}",
                        "opcode": "EventSemaphore",
                        "sync_info": {"on_update": [u], "on_wait": []},
                    })
            else:
                out.append(inst)
        b["instructions"] = out
        for sb in b.get("blocks", []):
            fix_block(sb)

    for fn in d.get("functions", []):
        for blk in fn.get("blocks", []):
            fix_block(blk)
    return json.dumps(d).encode()


def _install_drainfix():
    import concourse.bass as bass
    if getattr(bass.Bass, "_drainfix_installed", False):
        return
    orig = bass.Bass.to_json_bytes

    def patched(self):
        return _fix_module_json(orig(self))

    bass.Bass.to_json_bytes = patched
    bass.Bass._drainfix_installed = True


# ---------------------------------------------------------------- program
_NC_CACHE = []


def _build_nc(reps=1):
    import concourse.bass as bass
    import concourse.mybir as mybir
    from concourse.tile import TileContext
    from contextlib import ExitStack

    f32 = mybir.dt.float32
    f32r = mybir.dt.float32r
    bf16 = mybir.dt.bfloat16
    f8 = mybir.dt.float8e4
    EXP = mybir.ActivationFunctionType.Exp

    nc = bass.Bass("TRN2", num_devices=NCORES)

    xqT = nc.dram_tensor("xqT", [D, S], f32, kind="ExternalInput")
    xkT = nc.dram_tensor("xkT", [D, S], f32, kind="ExternalInput")
    xvT = nc.dram_tensor("xvT", [D, S], f32, kind="ExternalInput")
    wq = nc.dram_tensor("wq", [D, CPC], f32, kind="ExternalInput")
    wk = nc.dram_tensor("wk", [D, CPC], f32, kind="ExternalInput")
    wv = nc.dram_tensor("wv", [D, CPC], f32, kind="ExternalInput")
    bqd = nc.dram_tensor("bq", [CPC], f32, kind="ExternalInput")
    bkd = nc.dram_tensor("bk", [CPC], f32, kind="ExternalInput")
    bvd = nc.dram_tensor("bv", [CPC], f32, kind="ExternalInput")
    conesd = nc.dram_tensor("cones", [1, 64], f32, kind="ExternalInput")
    outd = nc.dram_tensor("out", [CPC, S], f32, kind="ExternalOutput")

    with ExitStack() as ctx:
        ctx.enter_context(nc.allow_low_precision(
            reason="f32r tiles are full fp32 storage; matmul accumulates f32; "
                   "bf16 q/k/alpha within tolerance"))
        tc = ctx.enter_context(TileContext(nc))
        sb = ctx.enter_context(tc.tile_pool(name="sb", bufs=1))
        ps = ctx.enter_context(tc.tile_pool(name="ps", bufs=1, space="PSUM"))

        # ---- constants ----
        bq_sb = sb.tile([128, NP], f32, name="bq_sb")
        nc.sync.dma_start(out=bq_sb, in_=bqd.rearrange("(p c) -> c p", p=NP))
        bk_sb = sb.tile([128, NP], f32, name="bk_sb")
        nc.sync.dma_start(out=bk_sb, in_=bkd.rearrange("(p c) -> c p", p=NP))
        bv_bc = sb.tile([128, CPC], f32, name="bv_bc")
        _bva = bvd[:]
        nc.sync.dma_start(
            out=bv_bc,
            in_=bass.AP(tensor=_bva.tensor, offset=_bva.offset,
                        ap=[[0, 128]] + list(_bva.ap)))
        ones = sb.tile([1, 64], f32r, name="ones")
        nc.sync.dma_start(out=ones, in_=conesd[:, :].bitcast(f32r))

        # resident weights [128, dd, 512]; loaded in dd-halves so the first
        # projection matmuls can start after half a weight transfer.  wq is
        # loaded up front; wk/wv loads are emitted inside the schedule after
        # the first x-chunk DMA so they don't delay it in the queue.
        wq_sb = sb.tile([128, NDT, CPC], f32r, name="wq_sb")
        wk_sb = sb.tile([128, NDT, CPC], f32r, name="wk_sb")
        wv_sb = sb.tile([128, NDT, CPC], f32r, name="wv_sb")

        def load_w(w_sb, w_dram):
            ap = w_dram.rearrange("(dd p) c -> p dd c", p=128).bitcast(f32r)
            half = NDT // 2
            nc.sync.dma_start(out=w_sb[:, 0:half, :], in_=ap[:, 0:half, :])
            nc.sync.dma_start(out=w_sb[:, half:NDT, :], in_=ap[:, half:NDT, :])

        load_w(wq_sb, wq)

        # resident qT/kT [c 128, s 2048] per head pair, bf16
        qT = [sb.tile([128, S], bf16, name=f"qT{p}") for p in range(NP)]
        kT = [sb.tile([128, S], bf16, name=f"kT{p}") for p in range(NP)]
        # V augmented with ones column, bf16: [sk_in_tile, sk_tile, head, 65]
        v_aug = sb.tile([128, NSK, 8, 65], bf16, name="v_aug")
        nc.gpsimd.memset(v_aug[:, :, :, 64:65], 1.0)

        # ---- helpers ----
        def emit_proj_qk(t, w_sb, x_dram, bias_sb, dst, after_first_dma=None):
            """dst[p][:, t*512:(t+1)*512] = (x @ W + b).T chunk for all p.
            x chunk loaded ONCE in two fused 4-dtile DMAs, W resident."""
            half = NDT // 2
            xap = x_dram.rearrange("(dd p) s -> p dd s", p=128)
            xts = []
            for hh in range(2):
                x_t = sb.tile([128, half, SQC], f32r, tag="xs", bufs=4,
                              name=f"x_{t}_{hh}")
                nc.sync.dma_start(
                    out=x_t,
                    in_=xap[:, hh * half:(hh + 1) * half,
                            t * SQC:(t + 1) * SQC].bitcast(f32r))
                xts.append(x_t)
                if hh == 1 and after_first_dma is not None:
                    after_first_dma()
            for p in range(NP):
                pr = ps.tile([128, SQC], f32, tag="vp", bufs=2, name=f"prj_{t}_{p}")
                for dd in range(NDT):
                    nc.tensor.matmul(
                        pr[:, :],
                        w_sb[:, dd, p * 128:(p + 1) * 128],
                        xts[dd // half][:, dd % half, :],
                        start=(dd == 0), stop=(dd == NDT - 1))
                nc.vector.tensor_scalar_add(
                    dst[p][:, t * SQC:(t + 1) * SQC], pr[:, :], bias_sb[:, p:p + 1])

        def emit_proj_v(tt_list):
            """v_aug[:, tt, h, 0:64] = (xv @ Wv + bv) rows tt*128.., bf16."""
            for tt in tt_list:
                xv_t = sb.tile([128, NDT, 128], f32r, tag="xv", bufs=2,
                               name=f"xv_{tt}")
                nc.sync.dma_start(
                    out=xv_t,
                    in_=xvT.rearrange("(dd p) s -> p dd s", p=128)
                    [:, :, tt * 128:(tt + 1) * 128].bitcast(f32r))
                pv = ps.tile([128, CPC], f32, tag="vp", bufs=2, name=f"pv_{tt}")
                for dd in range(NDT):
                    nc.tensor.matmul(
                        pv[:, :],
                        xv_t[:, dd, :],
                        wv_sb[:, dd, :],
                        start=(dd == 0), stop=(dd == NDT - 1))
                nc.vector.tensor_add(
                    v_aug[:, tt, :, 0:64],
                    pv.rearrange("c (h d) -> c h d", h=8),
                    bv_bc.rearrange("c (h d) -> c h d", h=8))

        alpha = {}  # (p, j, i) -> tile [128, 2, 512] bf16 ((h, sq) of sk-tile i)

        def emit_qkexp(p, j, ibs):
            """scoresT + exp for pair p, sq chunk j, sk-tiles ibs.  One PSUM
            tile and ONE activation per sk-tile (both heads packed) so the
            scores pool sustains two sk-tiles in flight."""
            for i in ibs:
                sc = ps.tile([128, 2, SQC], f32, tag="scores", bufs=2,
                             name=f"sc_{p}_{j}_{i}")
                for h in range(2):
                    nc.tensor.matmul(
                        sc[:, h, :],
                        kT[p][h * 64:(h + 1) * 64, i * 128:(i + 1) * 128],
                        qT[p][h * 64:(h + 1) * 64, j * SQC:(j + 1) * SQC],
                        start=True, stop=True)
                a_t = sb.tile([128, 2, SQC], bf16, tag="alpha", bufs=26,
                              name=f"al_{p}_{j}_{i}")
                nc.scalar.activation(a_t[:, :, :], sc[:, :, :], EXP, scale=0.125)
                alpha[(p, j, i)] = a_t

        def emit_av_norm(p, j, h, av):
            """Normalize + store one head-of-pair ctxT block.  The recip of
            the denominator row is broadcast across partitions on the (idle)
            GPSIMD engine instead of a PE ones-matmul + PSUM round-trip."""
            rec = sb.tile([1, SQC], f32, tag="rec", bufs=4, name=f"rec_{p}_{j}_{h}")
            nc.vector.reciprocal(rec[:, :], av[64:65, :])
            bcs = sb.tile([64, SQC], f32, tag="bcs", bufs=2, name=f"bcs_{p}_{j}_{h}")
            nc.gpsimd.partition_broadcast(bcs[:, :], rec[:, :])
            cx = sb.tile([64, SQC], f32, tag="cx", bufs=2, name=f"cx_{p}_{j}_{h}")
            nc.vector.tensor_mul(cx[:, :], av[0:64, :], bcs[:, :])
            r0 = (2 * p + h) * 64
            nc.sync.dma_start(
                out=outd[r0:r0 + 64, j * SQC:(j + 1) * SQC], in_=cx[:, :])

        def emit_av_slices(p, j):
            """8 callables, each emitting 4 A*V matmuls of (p,j) — slice s
            covers h=s//4, sk-tiles 4*(s%4)..4*(s%4)+3.  Interleaved between
            qkexp blocks so ACT never starves while PE runs AV."""
            tiles = {}

            def slice_fn(s):
                q4, h = divmod(s, 2)
                if q4 == 0:
                    tiles[h] = ps.tile([65, SQC], f32, tag="av", bufs=2,
                                       name=f"av_{p}_{j}_{h}")
                av = tiles[h]
                for i in range(4 * q4, 4 * q4 + 4):
                    a_t = alpha[(p, j, i)] if h == 0 else alpha.pop((p, j, i))
                    nc.tensor.matmul(
                        av[:, :],
                        v_aug[:, i, 2 * p + h, :],
                        a_t[:, h, :],
                        start=(i == 0), stop=(i == NSK - 1))
                if q4 == 3:
                    emit_av_norm(p, j, h, av)

            return [lambda s=s: slice_fn(s) for s in range(8)]

        def emit_av(p, j):
            for fn in emit_av_slices(p, j):
                fn()

        def emit_step(p, j, av_sl):
            """One steady-state step: qkexp sk-tile blocks of (p,j)
            interleaved with the AV slices of the previous (p,j)."""
            for i in range(NSK):
                emit_qkexp(p, j, [i])
                if av_sl is not None and i % 2 == 1:
                    av_sl[i // 2]()

        # ---- emission schedule ----
        # qkexp(p, j, [g]) needs kT[p] cols (2g+2)*128 <= produced K chunks
        # (K chunk t covers cols t*512:(t+1)*512, i.e. g-pairs 2t, 2t+1) and
        # qT[p] chunk j.  av(p, j) needs all alpha(p, j, *) and full v_aug.
        did_load_w = [False]

        def _emit_all():
            first = not did_load_w[0]
            did_load_w[0] = True
            cb = (lambda: load_w(wk_sb, wk)) if first else None
            # Head: DMA queue order wq, x0, wk, xk0..xk2, wv, xv0-3, xk3,
            # xv4-15, xq1-3.  Each K chunk t enables qkexp(p, 0, 4t..4t+3)
            # for ALL p, so ACT runs continuously from the first exp on.
            # Head: sets (0,0) and (1,0) produced as K chunks land; V and
            # av(0,0) woven in so the alpha pool (26 bf16 tiles) never
            # overfills and ACT stays fed.
            emit_proj_qk(0, wq_sb, xqT, bq_sb, qT, after_first_dma=cb)
            emit_proj_qk(0, wk_sb, xkT, bk_sb, kT)
            emit_qkexp(0, 0, range(0, 4))
            emit_proj_qk(1, wk_sb, xkT, bk_sb, kT)
            if first:
                load_w(wv_sb, wv)
            emit_qkexp(0, 0, range(4, 8))
            emit_qkexp(1, 0, range(0, 4))
            emit_proj_v(range(0, 4))
            sl0 = emit_av_slices(0, 0)
            emit_proj_qk(2, wk_sb, xkT, bk_sb, kT)
            emit_qkexp(0, 0, range(8, 12))
            emit_qkexp(1, 0, range(4, 8))
            emit_proj_v(range(4, 8))
            emit_proj_qk(3, wk_sb, xkT, bk_sb, kT)
            emit_qkexp(0, 0, range(12, 16))          # (0,0) complete [24]
            sl0[0]()
            sl0[1]()                                 # pop i0..3  [20]
            emit_qkexp(1, 0, range(8, 12))           # [24]
            emit_proj_v(range(8, 12))
            sl0[2]()
            sl0[3]()                                 # pop i4..7  [20]
            emit_qkexp(1, 0, range(12, 16))          # (1,0) complete [24]
            emit_proj_v(range(12, 16))
            sl0[4]()
            sl0[5]()                                 # [20]
            sl0[6]()
            sl0[7]()                                 # [16]
            # transition + steady: each step retires the previous set.
            emit_step(2, 0, emit_av_slices(1, 0))
            emit_proj_qk(1, wq_sb, xqT, bq_sb, qT)   # qT chunk 1
            emit_step(3, 0, emit_av_slices(2, 0))
            steps = [(p, j) for j in range(1, NJ) for p in range(NP)]
            prev = [(3, 0)]
            for n, (p, j) in enumerate(steps):
                emit_step(p, j, emit_av_slices(*prev.pop(0)))
                prev.append((p, j))
                if n == 0:
                    emit_proj_qk(2, wq_sb, xqT, bq_sb, qT)   # qT chunk 2
                elif n == 4:
                    emit_proj_qk(3, wq_sb, xqT, bq_sb, qT)   # qT chunk 3
            for pj in prev:
                emit_av(*pj)

        for _rep in range(reps):
            _emit_all()

    return nc


_NC_BY_REPS = {}


def _get_nc(reps=1):
    if reps not in _NC_BY_REPS:
        _install_drainfix()
        _NC_BY_REPS[reps] = _build_nc(reps)
    return _NC_BY_REPS[reps]


# ---------------------------------------------------------------- entry
def kernel(query, key_in, value, Wq, bq, Wk, bk, Wv, bv):
    from concourse.bass_utils import run_bass_kernel_spmd

    nc = _get_nc()
    query = np.asarray(query, np.float32)
    key_in = np.asarray(key_in, np.float32)
    value = np.asarray(value, np.float32)
    Wq = np.asarray(Wq, np.float32)
    Wk = np.asarray(Wk, np.float32)
    Wv = np.asarray(Wv, np.float32)
    bq = np.asarray(bq, np.float32)
    bk = np.asarray(bk, np.float32)
    bv = np.asarray(bv, np.float32)

    in_maps = []
    for c in range(NCORES):
        b, hg = divmod(c, 2)
        cols = slice(hg * CPC, (hg + 1) * CPC)
        in_maps.append({
            "xqT": np.ascontiguousarray(query[b].T),
            "xkT": np.ascontiguousarray(key_in[b].T),
            "xvT": np.ascontiguousarray(value[b].T),
            "wq": np.ascontiguousarray(Wq[:, cols]),
            "wk": np.ascontiguousarray(Wk[:, cols]),
            "wv": np.ascontiguousarray(Wv[:, cols]),
            "bq": np.ascontiguousarray(bq[cols]),
            "bk": np.ascontiguousarray(bk[cols]),
            "bv": np.ascontiguousarray(bv[cols]),
            "cones": np.ones((1, 64), np.float32),
        })

    res = run_bass_kernel_spmd(nc, in_maps, core_ids=list(range(NCORES)))

    out = np.empty((B, S, D), np.float32)
    for c in range(NCORES):
        b, hg = divmod(c, 2)
        out[b, :, hg * CPC:(hg + 1) * CPC] = res.results[c]["out"].T
    return out

